# revision 52
# baseline (speedup 1.0000x reference)
"""Trainium2 Bass kernel for nn_MultiHeadAttention_38611755991513.

Reference computation (B=2, D=1024, L=2048, H=16, DK=64):
    q/k/v = conv1d(kernel=1) projections of query [B, D, L]
    att   = softmax(mask(q^T k / sqrt(DK)))   with key-only mask [B, 1, L]
    out   = Wo @ (att @ v heads recombined) + bo

Sharding: 32 (batch, head) pairs -> 4 heads (one batch) per core.
Each core computes its 4 heads' attention plus the partial O-projection
(Wo columns for its heads); the host sums the 4 partials per batch.

Key optimizations:
- Key-only mask -> masked keys compacted away on the host; the ragged last
  128-key tile overlaps the previous one so every tile is full width (the
  duplicated rows are dead: zeroed V rows and ones-column).
- Scores are computed transposed (S^T[k, q]) so exp(S^T) is directly the
  moving operand of att@v; the softmax denominator comes free as a 65th
  "ones" column of the V operand.
- Per (head-pair, key-tile) the two heads' scores land in ONE [128, 2, 512]
  PSUM tile (the K=64 matmuls target row groups 0-63 / 64-127 and stream
  concurrently when issued back-to-back) and are consumed by ONE merged
  exp, which frees both heads' slots simultaneously so the next pair is
  ready together -> the scheduler keeps pairs adjacent.
- exp thus runs as [128, 1024]-sized ACT instructions; the attention inner
  loop is paced by ACT while projections/O-chunks fill the PE.
- bv is folded into bo on the host (out = Wo@(y/den) + (Wo@bv + bo)), and
  bq is pre-scaled by 1/sqrt(DK).
"""

import sys

sys.path.insert(0, "/opt/trn_rl_repo")

import numpy as np
import ml_dtypes

import concourse.bass as bass
import concourse.tile as tile
from concourse import bacc, mybir
from concourse.bass_utils import run_bass_kernel_spmd

B, D, L, H = 2, 1024, 2048, 16
DK = 64
NCORES = 8
HPC = 4              # heads per core
DH = HPC * DK        # 256 head-dims per core
KT = D // 128        # 8 contraction tiles for the projections
BF16 = mybir.dt.bfloat16
F32 = mybir.dt.float32
NPBF16 = ml_dtypes.bfloat16

TRACE = False            # set True (e.g. from test.py) to capture a HW profile
LAST_EXEC_NS = None
LAST_RESULTS = None

QBW = 512                # query-block width (one PSUM bank per head slot)


def _chunks(total, size):
    out = []
    s = 0
    while s < total:
        w = min(size, total - s)
        out.append((s, w))
        s += w
    return out


def _key_tiles(L_c):
    """Full-width 128-key tiles covering [0, L_c); the last tile overlaps the
    previous one when L_c is ragged (its first MT*128-L_c rows are dead)."""
    MT = (L_c + 127) // 128
    mts = [(i * 128, 128) for i in range(MT - 1)]
    mts.append((L_c - 128, 128))
    return mts


def _build(L_c):
    """Build + compile the per-core Bass program for compacted key length L_c."""
    assert L_c >= 128
    nc = bacc.Bacc("TRN2", debug=False, num_devices=NCORES)
    mts = _key_tiles(L_c)
    MT = len(mts)
    deadw = MT * 128 - L_c   # dead leading rows of the (overlapped) tail tile
    EXP = mybir.ActivationFunctionType.Exp

    xb_d = nc.declare_dram_parameter("xb", [D, L], BF16, isOutput=False)
    xk_d = nc.declare_dram_parameter("xk", [D, L_c], BF16, isOutput=False)
    vs_d = nc.declare_dram_parameter("vsetup", [MT, 128, HPC, 65], BF16, isOutput=False)
    wq_d = nc.declare_dram_parameter("wq", [D, DH], BF16, isOutput=False)
    wk_d = nc.declare_dram_parameter("wk", [D, DH], BF16, isOutput=False)
    wv_d = nc.declare_dram_parameter("wv", [D, DH], BF16, isOutput=False)
    wo_d = nc.declare_dram_parameter("wo", [DH, D], BF16, isOutput=False)
    bias_d = nc.declare_dram_parameter("bias", [128, 4], F32, isOutput=False)
    out_d = nc.declare_dram_parameter("out", [D, L], BF16, isOutput=True)

    from contextlib import ExitStack
    with tile.TileContext(nc) as tc, ExitStack() as ctx:
        pers = ctx.enter_context(tc.tile_pool(name="pers", bufs=1))

        def ptile(shape, dtype, name):
            return pers.tile(shape, dtype, tag=name, name=name)

        # persistent SBUF tensors.  Multi-slot layout ([128, slot, cols]) so
        # each logical group loads with ONE batched DMA — the Sync engine
        # issues descriptors at ~600ns each, so DMA count is a real cost.
        xk_a = ptile([128, KT, L_c], BF16, "xk_a")
        xb_a = ptile([128, KT, L], BF16, "xb_a")
        wq_a = ptile([128, KT, DH], BF16, "wq_a")
        wk_a = ptile([128, KT, DH], BF16, "wk_a")
        wv_a = ptile([128, KT, DH], BF16, "wv_a")
        wo_a = ptile([128, 2, D], BF16, "wo_a")
        xk_t = [xk_a[:, i] for i in range(KT)]
        xb_t = [xb_a[:, i] for i in range(KT)]
        wq_t = [wq_a[:, i] for i in range(KT)]
        wk_t = [wk_a[:, i] for i in range(KT)]
        wv_t = [wv_a[:, i] for i in range(KT)]
        wo_t = [wo_a[:, i] for i in range(2)]
        bias_all = ptile([128, 4], F32, "bias_all")
        bq_t = [bias_all[:, 2 * i + 0:2 * i + 1] for i in range(2)]
        bk_t = [bias_all[:, 2 * i + 1:2 * i + 2] for i in range(2)]
        q_t = [ptile([128, L], BF16, f"q{i}") for i in range(2)]
        k_t = [ptile([128, L_c], BF16, f"k{i}") for i in range(2)]
        z_t = [ptile([128, L], BF16, f"z{i}") for i in range(2)]
        # V operand per key tile: [128, head, 65]; col 64 is the ones column
        # (denominator); vsetup pre-zeroes dead rows and sets the ones
        vs_a = ptile([128, MT, HPC, 65], BF16, "vs_a")
        va_t = [vs_a[:, mt] for mt in range(MT)]
        # per-qblock output staging, so each qblock stores with ONE DMA
        ob_a = [ptile([128, 8, QBW], BF16, f"ob{i}") for i in range(2)]
        ones_t = ptile([1, 64], F32, "ones_t")

        def dram_slots(dram, n, cs=None, cw=None):
            ap = dram.rearrange("(t p) c -> p t c", p=128)
            return ap if cs is None else ap[:, :, cs:cs + cw]

        # input DMAs, critical-path order.  The first K chain chunk only needs
        # xk columns 0:128 plus wk, so load those thin slices first to get the
        # PE started early; then wq + the first xb slice (first Q chain), then
        # the xk remainder (K chains + V chains), then the rest.
        # DMA transfers packet-spray across all 16 engines (~290 GB/s
        # aggregate), so what matters is the BYTE count ahead of each
        # consumer.  Load exactly what the pipeline front needs first: the
        # kt=0 halves of wk/wq, xk's first key tile, and xb's first block.
        nc.sync.dma_start(xk_a[:, :, 0:128], dram_slots(xk_d, KT, 0, 128))
        nc.sync.dma_start(wk_a[:, :, 0:128], dram_slots(wk_d, KT)[:, :, 0:128])
        nc.sync.dma_start(wq_a[:, :, 0:128], dram_slots(wq_d, KT)[:, :, 0:128])
        nc.sync.dma_start(xb_a[:, :, 0:256], dram_slots(xb_d, KT, 0, 256))
        nc.sync.dma_start(xb_a[:, :, 256:512], dram_slots(xb_d, KT, 256, 256))
        nc.sync.dma_start(bias_all[:], bias_d[:])
        nc.sync.dma_start(
            xk_a[:, :, 128:512], dram_slots(xk_d, KT, 128, 384)
        )
        nc.sync.dma_start(wk_a[:, :, 128:256], dram_slots(wk_d, KT)[:, :, 128:256])
        nc.sync.dma_start(wq_a[:, :, 128:256], dram_slots(wq_d, KT)[:, :, 128:256])
        nc.sync.dma_start(
            xk_a[:, :, 512:L_c], dram_slots(xk_d, KT, 512, L_c - 512)
        )
        nc.sync.dma_start(wv_a[:], dram_slots(wv_d, KT))
        nc.sync.dma_start(vs_a[:], vs_d.rearrange("t p h c -> p t h c"))
        for js in (512, 1024, 1536):
            nc.sync.dma_start(xb_a[:, :, js:js + 512], dram_slots(xb_d, KT, js, 512))
        nc.sync.dma_start(wo_a[:], dram_slots(wo_d, 2))

        with (
            tc.tile_pool(name="psA", bufs=2, space="PSUM") as pa,
            tc.tile_pool(name="psY", bufs=2, space="PSUM") as pb,
            tc.tile_pool(name="psO", bufs=2, space="PSUM") as pox,
            tc.tile_pool(name="pexp", bufs=2 * MT + 4) as pp,
            tc.tile_pool(name="small", bufs=3) as psm,
        ):
            def k_chain(kt, ns, nw, halves=None):
                kp = pox.tile([128, nw], F32, tag="po", name=f"kp{kt}_{ns}")

                def half(lo, hi):
                    for kk in range(lo, hi):
                        nc.tensor.matmul(
                            kp[:],
                            wk_t[kk][:, kt * 128:(kt + 1) * 128],
                            xk_t[kk][:, ns:ns + nw],
                            start=(kk == 0), stop=(kk == KT - 1),
                        )
                    if hi == KT:
                        nc.vector.tensor_scalar_add(k_t[kt][:, ns:ns + nw], kp[:], bk_t[kt][:])
                if halves is None:
                    half(0, KT)
                else:
                    halves.append(lambda: half(0, KT // 2))
                    halves.append(lambda: half(KT // 2, KT))

            def q_chain(qs, kt, halves=None, jw=QBW):
                qp = pox.tile([128, jw], F32, tag="po", name=f"qp{kt}_{qs}")

                def half(lo, hi):
                    for kk in range(lo, hi):
                        nc.tensor.matmul(
                            qp[:],
                            wq_t[kk][:, kt * 128:(kt + 1) * 128],
                            xb_t[kk][:, qs:qs + jw],
                            start=(kk == 0), stop=(kk == KT - 1),
                        )
                    if hi == KT:
                        nc.vector.tensor_scalar_add(q_t[kt][:, qs:qs + jw], qp[:], bq_t[kt][:])
                if halves is None:
                    half(0, KT)
                else:
                    halves.append(lambda: half(0, KT // 2))
                    halves.append(lambda: half(KT // 2, KT))

            def v_chain(mt):
                ms, mw = mts[mt]
                vp = pox.tile([mw, DH], F32, tag="po", name=f"vp{mt}")
                for kk in range(KT):
                    nc.tensor.matmul(
                        vp[:],
                        xk_t[kk][:, ms:ms + mw],
                        wv_t[kk][:],
                        start=(kk == 0), stop=(kk == KT - 1),
                    )
                for h in range(HPC):
                    nc.vector.tensor_copy(va_t[mt][:, h, 0:64], vp[:, h * 64:(h + 1) * 64])
                if mt == MT - 1 and deadw:
                    # re-zero the dead overlap rows the copy just filled
                    nc.vector.memset(va_t[mt][0:deadw, :, 0:64], 0)

            def o_chunk(qi, qs, m8):
                op = pox.tile([128, QBW], F32, tag="po", name=f"o{qs}_{m8}")
                for kt in range(2):
                    nc.tensor.matmul(
                        op[:],
                        wo_t[kt][:, m8 * 128:(m8 + 1) * 128],
                        z_t[kt][:, qs:qs + QBW],
                        start=(kt == 0), stop=(kt == 1),
                    )
                nc.vector.tensor_copy(ob_a[qi % 2][:, m8, :], op[:])

            def o_flush(qi, qs, lo=0, hi=8):
                nc.sync.dma_start(
                    out_d.rearrange("(t p) c -> p t c", p=128)[:, lo:hi, qs:qs + QBW],
                    ob_a[qi % 2][:, lo:hi],
                )

            # minimal prologue: a small first K chunk plus the first Q block
            if L_c <= 128:
                kchunks = [(0, L_c)]
            elif L_c <= 512:
                kchunks = [(0, 128), (128, L_c - 128)]
            else:
                kchunks = [(0, 128), (128, 384)] + _chunks(L_c, 512)[1:]
            nc.vector.memset(ones_t[:], 1.0)
            k_chain(0, *kchunks[0])
            # HAM warmup: keep the PE streaming on already-loaded data while
            # the first Q-block DMAs land, so the clock is at 2.4 GHz (not the
            # idle-throttled 1.2) when the real pipeline starts
            for w in range(3):
                wu = pox.tile([128, 128], F32, tag="po", name=f"wu{w}")
                for kk in range(KT):
                    nc.tensor.matmul(
                        wu[:],
                        wk_t[kk][:, 0:128],
                        xk_t[kk][:, 0:128],
                        start=(kk == 0), stop=(kk == KT - 1),
                    )
            q_chain(0, 0, jw=256)
            q_chain(256, 0, jw=256)

            # ---- software-pipelined attention, head-PAIR phases ----
            # Per key tile: both heads' K=64 score matmuls write one
            # [128, 2, 512] PSUM tile (row groups 0-63 / 64-127, concurrent
            # when adjacent), then one merged exp -> [128, 2, 512] bf16.
            def y_head(h, qs, p_tiles, yq, pe_bcast=False):
                state = {}

                def y_mt(mt):
                    if mt == 0:
                        state["yp"] = pb.tile([65, QBW], F32, tag="y", name=f"y{qs}_{h}")
                    nc.tensor.matmul(
                        state["yp"],
                        va_t[mt][:, h, :],
                        p_tiles[mt][:, h % 2, :],
                        start=(mt == 0), stop=(mt == MT - 1),
                    )

                def finish():
                    pt, off = h // 2, (h % 2) * 64
                    yp = state["yp"]
                    rt = psm.tile([1, QBW], F32, tag="rrow", name=f"rt{qs}_{h}")
                    nc.vector.tensor_copy(rt[:], yp[64:65, :])
                    rc = psm.tile([1, QBW], F32, tag="recip", name=f"rc{qs}_{h}")
                    nc.vector.reciprocal_approx_fast(rc[:], rt[:])
                    rb = psm.tile([64, QBW], F32, tag="rb", name=f"rb{qs}_{h}")
                    nc.gpsimd.partition_broadcast(rb[:], rc[:])
                    zsl = z_t[pt][off:off + 64, qs:qs + QBW]
                    nc.vector.tensor_mul(zsl, yp[0:64, :], rb[:])

                for mt in range(0, MT, 2):
                    def two(mt=mt):
                        y_mt(mt)
                        if mt + 1 < MT:
                            y_mt(mt + 1)
                    yq.append(two)
                yq.append(finish)

            fillers = []   # (cost, emit) pairs
            fi = 0

            def pop_fillers(budget):
                nonlocal fi
                while budget > 0 and fi < len(fillers):
                    cost, emit = fillers[fi]
                    emit()
                    fi += 1
                    budget -= cost
                return budget

            def pop_until(idx):
                nonlocal fi
                while fi < idx:
                    fillers[fi][1]()
                    fi += 1

            # K/Q/V chains are PREREQUISITES of later emissions: Tile derives
            # dependencies from emission order, so a consumer emitted before
            # its writer would silently read stale data.  k-halves go first
            # (phase-0 scores consume them tile by tile), then v_chains
            # (consumed by the y chains emitted at the end of phase 0), then
            # per-block q chains with recorded per-phase deadlines.
            halves = []
            for ns, nw in kchunks[1:]:
                k_chain(0, ns, nw, halves)
            for ns, nw in kchunks:
                k_chain(1, ns, nw, halves)
            fillers.extend((4, fn) for fn in halves)
            vk_deadline = len(fillers)
            deadline = {}
            for qi in range(len(_chunks(L, QBW))):
                for hp in range(2):
                    if (qi, hp) == (0, 0):
                        continue   # prologue chains
                    halves = []
                    q_chain(qi * QBW, hp, halves)
                    fillers.extend((4, fn) for fn in halves)
                deadline[(qi, 0)] = deadline[(qi, 1)] = len(fillers)

            qblocks = _chunks(L, QBW)
            yq = []       # pending y work units of the previous pair

            for qi, (qs, qw) in enumerate(qblocks):
                for hp in range(2):
                    hA, hB = 2 * hp, 2 * hp + 1
                    first_phase = (qi, hp) == (0, 0)
                    last_phase = (qi, hp) == (len(qblocks) - 1, 1)
                    pop_until(deadline.get((qi, hp), 0))   # q chains this phase reads
                    if hp == 1 and qi >= 1:
                        # z of block qi-1 completed during the previous phase:
                        # its O-projection chunks become filler work now.  In
                        # the last phase, hold half of them back so the PE has
                        # ready work during the final normalize chain.
                        pqs = qblocks[qi - 1][0]
                        nf = 4 if last_phase else 8
                        for m8 in range(nf):
                            fillers.append(
                                (4, lambda qi=qi, pqs=pqs, m8=m8: o_chunk(qi - 1, pqs, m8))
                            )
                        if nf == 8:
                            fillers.append((0, lambda qi=qi, pqs=pqs: o_flush(qi - 1, pqs)))
                    ptiles = []
                    for mt, (ms, mw) in enumerate(mts):
                        # fillers first: anything a later score matmul reads
                        # (K/Q chains) must already be emitted
                        for _ in range(2):
                            if yq:
                                yq.pop(0)()
                        if first_phase:
                            pop_fillers(8)
                            v_chain(mt)
                        else:
                            pop_fillers(4)
                        sp = pa.tile([128, 2, QBW], F32, tag="wide", name=f"s{qs}_{hp}_{mt}")
                        for sl, off in ((0, 0), (1, 64)):
                            nc.tensor.matmul(
                                sp[:, sl, :],
                                k_t[hp][off:off + 64, ms:ms + mw],
                                q_t[hp][off:off + 64, qs:qs + QBW],
                                start=True, stop=True,
                            )
                        px = pp.tile([128, 2, QBW], BF16, tag="p", name=f"p{qs}_{hp}_{mt}")
                        nc.scalar.activation(px[:], sp[:], EXP)
                        ptiles.append(px)
                    while yq:
                        yq.pop(0)()
                    yq = []
                    if first_phase:
                        pop_until(vk_deadline)   # v_chains feed the y units below
                    y_head(hA, qs, ptiles, yq, pe_bcast=last_phase)
                    y_head(hB, qs, ptiles, yq, pe_bcast=last_phase)

            # drain: final pair's y work, then the held-back previous-block O
            # chunks (ready immediately -- they cover the PE idle behind the
            # final normalize), then the last block's own O chunks
            qi = len(qblocks) - 1
            qs = qblocks[-1][0]
            pqs = qblocks[-2][0]
            while yq:
                yq.pop(0)()
            pop_fillers(1000)
            for m8 in range(4, 8):
                o_chunk(qi - 1, pqs, m8)
            o_flush(qi - 1, pqs)
            for m8 in range(4):
                o_chunk(qi, qs, m8)
            o_flush(qi, qs, 0, 4)   # first half streams out while the rest computes
            for m8 in range(4, 8):
                o_chunk(qi, qs, m8)
            o_flush(qi, qs, 4, 8)

    nc.compile()
    return nc


_NC_CACHE = {}


def _get_nc(L_c):
    if L_c not in _NC_CACHE:
        _NC_CACHE[L_c] = _build(L_c)
    return _NC_CACHE[L_c]


def _install_ntff_hook():
    """Synthesize antenv.axon_hooks (missing in this image) so trace=True works."""
    import types

    if "antenv.axon_hooks" in sys.modules:
        return
    try:
        if "/root/.axon_site" not in sys.path:
            sys.path.insert(0, "/root/.axon_site")
        from trn_agent_boot.trn_boot import _ntff_profile_via_ctypes

        hook = _ntff_profile_via_ctypes("/opt/axon/libaxon_pjrt.so")
        mod = types.ModuleType("antenv.axon_hooks")
        mod.get_axon_ntff_profile_hook = lambda: hook
        import antenv  # noqa: F401

        sys.modules["antenv.axon_hooks"] = mod
    except Exception:
        pass


def kernel(query, att_mask, Wq, bq, Wk, bk, Wv, bv, Wo, bo):
    global LAST_EXEC_NS, LAST_RESULTS
    query = np.asarray(query, dtype=np.float32)
    mask = np.asarray(att_mask).astype(bool).reshape(B, L)
    Wq, bq = np.asarray(Wq, np.float32), np.asarray(bq, np.float32)
    Wk, bk = np.asarray(Wk, np.float32), np.asarray(bk, np.float32)
    Wv, bv = np.asarray(Wv, np.float32), np.asarray(bv, np.float32)
    Wo, bo = np.asarray(Wo, np.float32), np.asarray(bo, np.float32)

    valid = [np.nonzero(mask[b])[0] for b in range(B)]
    L_c = max(len(v) for v in valid)
    out = np.empty((B, D, L), np.float32)
    if L_c == 0:
        out[:] = bo[None, :, None]
        return out

    scale = np.float32(1.0 / np.sqrt(DK))
    L_c = max(128, L_c)
    mts = _key_tiles(L_c)
    MT = len(mts)
    deadw = MT * 128 - L_c
    # per-batch compacted keys + V-operand init image (zeros, with the
    # ones/denominator column set on live rows only)
    xk_b, vs_b, xb_b = [], [], []
    for b in range(B):
        idx = valid[b]
        xk = np.zeros((D, L_c), np.float32)
        xk[:, :len(idx)] = query[b][:, idx]
        xk_b.append(xk.astype(NPBF16))
        vs = np.zeros((MT, 128, HPC, 65), np.float32)
        for t, (ms, mw) in enumerate(mts):
            live = (ms + np.arange(128)) < len(idx)
            if t == MT - 1:
                live &= np.arange(128) >= deadw
            vs[t, :, :, 64] = live[:, None].astype(np.float32)
        vs_b.append(vs.astype(NPBF16))
        xb_b.append(query[b].astype(NPBF16))

    in_maps = []
    for c in range(NCORES):
        b, g = divmod(c, NCORES // B)
        sl = slice(g * DH, (g + 1) * DH)
        in_maps.append({
            "xb": xb_b[b],
            "xk": xk_b[b],
            "vsetup": vs_b[b],
            "wq": np.ascontiguousarray((Wq[sl, :] * scale).T).astype(NPBF16),
            "wk": np.ascontiguousarray(Wk[sl, :].T).astype(NPBF16),
            "wv": np.ascontiguousarray(Wv[sl, :].T).astype(NPBF16),
            "wo": np.ascontiguousarray(Wo[:, sl].T).astype(NPBF16),
            "bias": np.stack(
                [(bq[sl] * scale), bk[sl]], axis=-1
            ).reshape(2, 128, 2).transpose(1, 0, 2).reshape(128, 4).astype(np.float32),
        })

    nc = _get_nc(L_c)
    if TRACE:
        _install_ntff_hook()
    res = run_bass_kernel_spmd(nc, in_maps, core_ids=list(range(NCORES)), trace=TRACE)
    LAST_EXEC_NS = res.exec_time_ns
    LAST_RESULTS = res

    bo_eff = (Wo @ bv + bo)[:, None]   # bv folded through the O projection
    parts = [res.results[c]["out"] for c in range(NCORES)]
    for b in range(B):
        if len(valid[b]) == 0:
            out[b] = bo[:, None]
        else:
            acc = parts[4 * b].astype(np.float32)
            for g in range(1, 4):
                acc = acc + parts[4 * b + g]
            out[b] = acc + bo_eff
    return out


# revision 53
# speedup vs baseline: 1.0313x; 1.0313x over previous
"""Trainium2 Bass kernel for nn_MultiHeadAttention_38611755991513.

Reference computation (B=2, D=1024, L=2048, H=16, DK=64):
    q/k/v = conv1d(kernel=1) projections of query [B, D, L]
    att   = softmax(mask(q^T k / sqrt(DK)))   with key-only mask [B, 1, L]
    out   = Wo @ (att @ v heads recombined) + bo

Sharding: 32 (batch, head) pairs -> 4 heads (one batch) per core.
Each core computes its 4 heads' attention plus the partial O-projection
(Wo columns for its heads); the host sums the 4 partials per batch.

Key optimizations:
- Key-only mask -> masked keys compacted away on the host; the ragged last
  128-key tile overlaps the previous one so every tile is full width (the
  duplicated rows are dead: zeroed V rows and ones-column).
- Scores are computed transposed (S^T[k, q]) so exp(S^T) is directly the
  moving operand of att@v; the softmax denominator comes free as a 65th
  "ones" column of the V operand.
- Per (head-pair, key-tile) the two heads' scores land in ONE [128, 2, 512]
  PSUM tile (the K=64 matmuls target row groups 0-63 / 64-127 and stream
  concurrently when issued back-to-back) and are consumed by ONE merged
  exp, which frees both heads' slots simultaneously so the next pair is
  ready together -> the scheduler keeps pairs adjacent.
- exp thus runs as [128, 1024]-sized ACT instructions; the attention inner
  loop is paced by ACT while projections/O-chunks fill the PE.
- bv is folded into bo on the host (out = Wo@(y/den) + (Wo@bv + bo)), and
  bq is pre-scaled by 1/sqrt(DK).
"""

import sys

sys.path.insert(0, "/opt/trn_rl_repo")

import numpy as np
import ml_dtypes

import concourse.bass as bass
import concourse.tile as tile
from concourse import bacc, mybir
from concourse.bass_utils import run_bass_kernel_spmd

B, D, L, H = 2, 1024, 2048, 16
DK = 64
NCORES = 8
HPC = 4              # heads per core
DH = HPC * DK        # 256 head-dims per core
KT = D // 128        # 8 contraction tiles for the projections
BF16 = mybir.dt.bfloat16
F32 = mybir.dt.float32
NPBF16 = ml_dtypes.bfloat16

TRACE = False            # set True (e.g. from test.py) to capture a HW profile
LAST_EXEC_NS = None
LAST_RESULTS = None

QBW = 512                # query-block width (one PSUM bank per head slot)


def _chunks(total, size):
    out = []
    s = 0
    while s < total:
        w = min(size, total - s)
        out.append((s, w))
        s += w
    return out


def _key_tiles(L_c):
    """Full-width 128-key tiles covering [0, L_c); the last tile overlaps the
    previous one when L_c is ragged (its first MT*128-L_c rows are dead)."""
    MT = (L_c + 127) // 128
    mts = [(i * 128, 128) for i in range(MT - 1)]
    mts.append((L_c - 128, 128))
    return mts


def _build(L_c):
    """Build + compile the per-core Bass program for compacted key length L_c."""
    assert L_c >= 128
    nc = bacc.Bacc("TRN2", debug=False, num_devices=NCORES)
    mts = _key_tiles(L_c)
    MT = len(mts)
    deadw = MT * 128 - L_c   # dead leading rows of the (overlapped) tail tile
    EXP = mybir.ActivationFunctionType.Exp

    xb_d = nc.declare_dram_parameter("xb", [D, L], BF16, isOutput=False)
    xk_d = nc.declare_dram_parameter("xk", [D, L_c], BF16, isOutput=False)
    vs_d = nc.declare_dram_parameter("vsetup", [MT, 128, HPC, 65], BF16, isOutput=False)
    wq_d = nc.declare_dram_parameter("wq", [D, DH], BF16, isOutput=False)
    wk_d = nc.declare_dram_parameter("wk", [D, DH], BF16, isOutput=False)
    wv_d = nc.declare_dram_parameter("wv", [D, DH], BF16, isOutput=False)
    wo_d = nc.declare_dram_parameter("wo", [DH, D], BF16, isOutput=False)
    bias_d = nc.declare_dram_parameter("bias", [128, 4], F32, isOutput=False)
    out_d = nc.declare_dram_parameter("out", [D, L], BF16, isOutput=True)

    from contextlib import ExitStack
    with tile.TileContext(nc) as tc, ExitStack() as ctx:
        pers = ctx.enter_context(tc.tile_pool(name="pers", bufs=1))

        def ptile(shape, dtype, name):
            return pers.tile(shape, dtype, tag=name, name=name)

        # persistent SBUF tensors.  Multi-slot layout ([128, slot, cols]) so
        # each logical group loads with ONE batched DMA — the Sync engine
        # issues descriptors at ~600ns each, so DMA count is a real cost.
        xk_a = ptile([128, KT, L_c], BF16, "xk_a")
        xb_a = ptile([128, KT, L], BF16, "xb_a")
        wq_a = ptile([128, KT, DH], BF16, "wq_a")
        wk_a = ptile([128, KT, DH], BF16, "wk_a")
        wv_a = ptile([128, KT, DH], BF16, "wv_a")
        wo_a = ptile([128, 2, D], BF16, "wo_a")
        xk_t = [xk_a[:, i] for i in range(KT)]
        xb_t = [xb_a[:, i] for i in range(KT)]
        wq_t = [wq_a[:, i] for i in range(KT)]
        wk_t = [wk_a[:, i] for i in range(KT)]
        wv_t = [wv_a[:, i] for i in range(KT)]
        wo_t = [wo_a[:, i] for i in range(2)]
        bias_all = ptile([128, 4], F32, "bias_all")
        bq_t = [bias_all[:, 2 * i + 0:2 * i + 1] for i in range(2)]
        bk_t = [bias_all[:, 2 * i + 1:2 * i + 2] for i in range(2)]
        q_t = [ptile([128, L], BF16, f"q{i}") for i in range(2)]
        k_t = [ptile([128, L_c], BF16, f"k{i}") for i in range(2)]
        z_t = [ptile([128, L], BF16, f"z{i}") for i in range(2)]
        # V operand per key tile: [128, head, 65]; col 64 is the ones column
        # (denominator); vsetup pre-zeroes dead rows and sets the ones
        vs_a = ptile([128, MT, HPC, 65], BF16, "vs_a")
        va_t = [vs_a[:, mt] for mt in range(MT)]
        # per-qblock output staging, so each qblock stores with ONE DMA
        ob_a = [ptile([128, 8, QBW], BF16, f"ob{i}") for i in range(2)]
        ones_t = ptile([1, 64], F32, "ones_t")

        def dram_slots(dram, n, cs=None, cw=None):
            ap = dram.rearrange("(t p) c -> p t c", p=128)
            return ap if cs is None else ap[:, :, cs:cs + cw]

        # input DMAs, critical-path order.  The first K chain chunk only needs
        # xk columns 0:128 plus wk, so load those thin slices first to get the
        # PE started early; then wq + the first xb slice (first Q chain), then
        # the xk remainder (K chains + V chains), then the rest.
        # DMA transfers packet-spray across all 16 engines (~290 GB/s
        # aggregate), so what matters is the BYTE count ahead of each
        # consumer.  Load exactly what the pipeline front needs first: the
        # kt=0 halves of wk/wq, xk's first key tile, and xb's first block.
        nc.sync.dma_start(xk_a[:, :, 0:128], dram_slots(xk_d, KT, 0, 128))
        nc.sync.dma_start(wk_a[:, :, 0:128], dram_slots(wk_d, KT)[:, :, 0:128])
        nc.sync.dma_start(wq_a[:, :, 0:128], dram_slots(wq_d, KT)[:, :, 0:128])
        nc.sync.dma_start(xb_a[:, :, 0:256], dram_slots(xb_d, KT, 0, 256))
        nc.sync.dma_start(xb_a[:, :, 256:512], dram_slots(xb_d, KT, 256, 256))
        nc.sync.dma_start(bias_all[:], bias_d[:])
        nc.sync.dma_start(
            xk_a[:, :, 128:512], dram_slots(xk_d, KT, 128, 384)
        )
        nc.sync.dma_start(wk_a[:, :, 128:256], dram_slots(wk_d, KT)[:, :, 128:256])
        nc.sync.dma_start(wq_a[:, :, 128:256], dram_slots(wq_d, KT)[:, :, 128:256])
        nc.sync.dma_start(
            xk_a[:, :, 512:L_c], dram_slots(xk_d, KT, 512, L_c - 512)
        )
        nc.sync.dma_start(wv_a[:], dram_slots(wv_d, KT))
        nc.sync.dma_start(vs_a[:], vs_d.rearrange("t p h c -> p t h c"))
        for js in (512, 1024, 1536):
            nc.sync.dma_start(xb_a[:, :, js:js + 512], dram_slots(xb_d, KT, js, 512))
        nc.sync.dma_start(wo_a[:], dram_slots(wo_d, 2))

        with (
            tc.tile_pool(name="psA", bufs=2, space="PSUM") as pa,
            tc.tile_pool(name="psY", bufs=2, space="PSUM") as pb,
            tc.tile_pool(name="psO", bufs=2, space="PSUM") as pox,
            tc.tile_pool(name="pexp", bufs=2 * MT + 4) as pp,
            tc.tile_pool(name="small", bufs=3) as psm,
        ):
            def k_chain(kt, ns, nw, halves=None):
                kp = pox.tile([128, nw], F32, tag="po", name=f"kp{kt}_{ns}")

                def half(lo, hi):
                    for kk in range(lo, hi):
                        nc.tensor.matmul(
                            kp[:],
                            wk_t[kk][:, kt * 128:(kt + 1) * 128],
                            xk_t[kk][:, ns:ns + nw],
                            start=(kk == 0), stop=(kk == KT - 1),
                        )
                    if hi == KT:
                        nc.vector.tensor_scalar_add(k_t[kt][:, ns:ns + nw], kp[:], bk_t[kt][:])
                if halves is None:
                    half(0, KT)
                else:
                    halves.append(lambda: half(0, KT // 2))
                    halves.append(lambda: half(KT // 2, KT))

            def q_chain(qs, kt, halves=None, jw=QBW):
                qp = pox.tile([128, jw], F32, tag="po", name=f"qp{kt}_{qs}")

                def half(lo, hi):
                    for kk in range(lo, hi):
                        nc.tensor.matmul(
                            qp[:],
                            wq_t[kk][:, kt * 128:(kt + 1) * 128],
                            xb_t[kk][:, qs:qs + jw],
                            start=(kk == 0), stop=(kk == KT - 1),
                        )
                    if hi == KT:
                        nc.vector.tensor_scalar_add(q_t[kt][:, qs:qs + jw], qp[:], bq_t[kt][:])
                if halves is None:
                    half(0, KT)
                else:
                    halves.append(lambda: half(0, KT // 2))
                    halves.append(lambda: half(KT // 2, KT))

            def v_chain(mt):
                ms, mw = mts[mt]
                vp = pox.tile([mw, DH], F32, tag="po", name=f"vp{mt}")
                for kk in range(KT):
                    nc.tensor.matmul(
                        vp[:],
                        xk_t[kk][:, ms:ms + mw],
                        wv_t[kk][:],
                        start=(kk == 0), stop=(kk == KT - 1),
                    )
                for h in range(HPC):
                    nc.vector.tensor_copy(va_t[mt][:, h, 0:64], vp[:, h * 64:(h + 1) * 64])
                if mt == MT - 1 and deadw:
                    # re-zero the dead overlap rows the copy just filled
                    nc.vector.memset(va_t[mt][0:deadw, :, 0:64], 0)

            def o_chunk(qi, qs, m8):
                op = pox.tile([128, QBW], F32, tag="po", name=f"o{qs}_{m8}")
                for kt in range(2):
                    nc.tensor.matmul(
                        op[:],
                        wo_t[kt][:, m8 * 128:(m8 + 1) * 128],
                        z_t[kt][:, qs:qs + QBW],
                        start=(kt == 0), stop=(kt == 1),
                    )
                nc.vector.tensor_copy(ob_a[qi % 2][:, m8, :], op[:])

            def o_flush(qi, qs, lo=0, hi=8):
                nc.sync.dma_start(
                    out_d.rearrange("(t p) c -> p t c", p=128)[:, lo:hi, qs:qs + QBW],
                    ob_a[qi % 2][:, lo:hi],
                )

            # minimal prologue: a small first K chunk plus the first Q block
            if L_c <= 128:
                kchunks = [(0, L_c)]
            elif L_c <= 512:
                kchunks = [(0, 128), (128, L_c - 128)]
            else:
                kchunks = [(0, 128), (128, 384)] + _chunks(L_c, 512)[1:]
            nc.vector.memset(ones_t[:], 1.0)
            k_chain(0, *kchunks[0])
            # HAM warmup: keep the PE streaming on already-loaded data while
            # the first Q-block DMAs land, so the clock is at 2.4 GHz (not the
            # idle-throttled 1.2) when the real pipeline starts
            for w in range(7):
                wu = pox.tile([128, 128], F32, tag="po", name=f"wu{w}")
                for kk in range(KT):
                    nc.tensor.matmul(
                        wu[:],
                        wk_t[kk][:, 0:128],
                        xk_t[kk][:, 0:128],
                        start=(kk == 0), stop=(kk == KT - 1),
                    )
            q_chain(0, 0, jw=256)
            q_chain(256, 0, jw=256)

            # ---- software-pipelined attention, head-PAIR phases ----
            # Per key tile: both heads' K=64 score matmuls write one
            # [128, 2, 512] PSUM tile (row groups 0-63 / 64-127, concurrent
            # when adjacent), then one merged exp -> [128, 2, 512] bf16.
            def y_head(h, qs, p_tiles, yq, pe_bcast=False):
                state = {}

                def y_mt(mt):
                    if mt == 0:
                        state["yp"] = pb.tile([65, QBW], F32, tag="y", name=f"y{qs}_{h}")
                    nc.tensor.matmul(
                        state["yp"],
                        va_t[mt][:, h, :],
                        p_tiles[mt][:, h % 2, :],
                        start=(mt == 0), stop=(mt == MT - 1),
                    )

                def finish():
                    pt, off = h // 2, (h % 2) * 64
                    yp = state["yp"]
                    rt = psm.tile([1, QBW], F32, tag="rrow", name=f"rt{qs}_{h}")
                    nc.vector.tensor_copy(rt[:], yp[64:65, :])
                    rc = psm.tile([1, QBW], F32, tag="recip", name=f"rc{qs}_{h}")
                    nc.vector.reciprocal_approx_fast(rc[:], rt[:])
                    rb = psm.tile([64, QBW], F32, tag="rb", name=f"rb{qs}_{h}")
                    nc.gpsimd.partition_broadcast(rb[:], rc[:])
                    zsl = z_t[pt][off:off + 64, qs:qs + QBW]
                    nc.vector.tensor_mul(zsl, yp[0:64, :], rb[:])

                for mt in range(0, MT, 2):
                    def two(mt=mt):
                        y_mt(mt)
                        if mt + 1 < MT:
                            y_mt(mt + 1)
                    yq.append(two)
                yq.append(finish)

            fillers = []   # (cost, emit) pairs
            fi = 0

            def pop_fillers(budget):
                nonlocal fi
                while budget > 0 and fi < len(fillers):
                    cost, emit = fillers[fi]
                    emit()
                    fi += 1
                    budget -= cost
                return budget

            def pop_until(idx):
                nonlocal fi
                while fi < idx:
                    fillers[fi][1]()
                    fi += 1

            # K/Q/V chains are PREREQUISITES of later emissions: Tile derives
            # dependencies from emission order, so a consumer emitted before
            # its writer would silently read stale data.  k-halves go first
            # (phase-0 scores consume them tile by tile), then v_chains
            # (consumed by the y chains emitted at the end of phase 0), then
            # per-block q chains with recorded per-phase deadlines.
            halves = []
            for ns, nw in kchunks[1:]:
                k_chain(0, ns, nw, halves)
            for ns, nw in kchunks:
                k_chain(1, ns, nw, halves)
            fillers.extend((4, fn) for fn in halves)
            vk_deadline = len(fillers)
            deadline = {}
            for qi in range(len(_chunks(L, QBW))):
                for hp in range(2):
                    if (qi, hp) == (0, 0):
                        continue   # prologue chains
                    halves = []
                    q_chain(qi * QBW, hp, halves)
                    fillers.extend((4, fn) for fn in halves)
                deadline[(qi, 0)] = deadline[(qi, 1)] = len(fillers)

            qblocks = _chunks(L, QBW)
            yq = []       # pending y work units of the previous pair

            for qi, (qs, qw) in enumerate(qblocks):
                for hp in range(2):
                    hA, hB = 2 * hp, 2 * hp + 1
                    first_phase = (qi, hp) == (0, 0)
                    last_phase = (qi, hp) == (len(qblocks) - 1, 1)
                    pop_until(deadline.get((qi, hp), 0))   # q chains this phase reads
                    if hp == 1 and qi >= 1:
                        # z of block qi-1 completed during the previous phase:
                        # its O-projection chunks become filler work now.  In
                        # the last phase, hold half of them back so the PE has
                        # ready work during the final normalize chain.
                        pqs = qblocks[qi - 1][0]
                        nf = 4 if last_phase else 8
                        for m8 in range(nf):
                            fillers.append(
                                (4, lambda qi=qi, pqs=pqs, m8=m8: o_chunk(qi - 1, pqs, m8))
                            )
                        if nf == 8:
                            fillers.append((0, lambda qi=qi, pqs=pqs: o_flush(qi - 1, pqs)))
                    ptiles = []
                    for mt, (ms, mw) in enumerate(mts):
                        # fillers first: anything a later score matmul reads
                        # (K/Q chains) must already be emitted
                        for _ in range(2):
                            if yq:
                                yq.pop(0)()
                        if first_phase:
                            pop_fillers(8)
                            v_chain(mt)
                        else:
                            pop_fillers(4)
                        sp = pa.tile([128, 2, QBW], F32, tag="wide", name=f"s{qs}_{hp}_{mt}")
                        for sl, off in ((0, 0), (1, 64)):
                            nc.tensor.matmul(
                                sp[:, sl, :],
                                k_t[hp][off:off + 64, ms:ms + mw],
                                q_t[hp][off:off + 64, qs:qs + QBW],
                                start=True, stop=True,
                            )
                        px = pp.tile([128, 2, QBW], BF16, tag="p", name=f"p{qs}_{hp}_{mt}")
                        nc.scalar.activation(px[:], sp[:], EXP)
                        ptiles.append(px)
                    while yq:
                        yq.pop(0)()
                    yq = []
                    if first_phase:
                        pop_until(vk_deadline)   # v_chains feed the y units below
                    y_head(hA, qs, ptiles, yq, pe_bcast=last_phase)
                    y_head(hB, qs, ptiles, yq, pe_bcast=last_phase)

            # drain: final pair's y work, then the held-back previous-block O
            # chunks (ready immediately -- they cover the PE idle behind the
            # final normalize), then the last block's own O chunks
            qi = len(qblocks) - 1
            qs = qblocks[-1][0]
            pqs = qblocks[-2][0]
            while yq:
                yq.pop(0)()
            pop_fillers(1000)
            for m8 in range(4, 8):
                o_chunk(qi - 1, pqs, m8)
            o_flush(qi - 1, pqs)
            for m8 in range(4):
                o_chunk(qi, qs, m8)
            o_flush(qi, qs, 0, 4)   # first half streams out while the rest computes
            for m8 in range(4, 8):
                o_chunk(qi, qs, m8)
            o_flush(qi, qs, 4, 8)

    nc.compile()
    return nc


_NC_CACHE = {}


def _get_nc(L_c):
    if L_c not in _NC_CACHE:
        _NC_CACHE[L_c] = _build(L_c)
    return _NC_CACHE[L_c]


def _install_ntff_hook():
    """Synthesize antenv.axon_hooks (missing in this image) so trace=True works."""
    import types

    if "antenv.axon_hooks" in sys.modules:
        return
    try:
        if "/root/.axon_site" not in sys.path:
            sys.path.insert(0, "/root/.axon_site")
        from trn_agent_boot.trn_boot import _ntff_profile_via_ctypes

        hook = _ntff_profile_via_ctypes("/opt/axon/libaxon_pjrt.so")
        mod = types.ModuleType("antenv.axon_hooks")
        mod.get_axon_ntff_profile_hook = lambda: hook
        import antenv  # noqa: F401

        sys.modules["antenv.axon_hooks"] = mod
    except Exception:
        pass


def kernel(query, att_mask, Wq, bq, Wk, bk, Wv, bv, Wo, bo):
    global LAST_EXEC_NS, LAST_RESULTS
    query = np.asarray(query, dtype=np.float32)
    mask = np.asarray(att_mask).astype(bool).reshape(B, L)
    Wq, bq = np.asarray(Wq, np.float32), np.asarray(bq, np.float32)
    Wk, bk = np.asarray(Wk, np.float32), np.asarray(bk, np.float32)
    Wv, bv = np.asarray(Wv, np.float32), np.asarray(bv, np.float32)
    Wo, bo = np.asarray(Wo, np.float32), np.asarray(bo, np.float32)

    valid = [np.nonzero(mask[b])[0] for b in range(B)]
    L_c = max(len(v) for v in valid)
    out = np.empty((B, D, L), np.float32)
    if L_c == 0:
        out[:] = bo[None, :, None]
        return out

    scale = np.float32(1.0 / np.sqrt(DK))
    L_c = max(128, L_c)
    mts = _key_tiles(L_c)
    MT = len(mts)
    deadw = MT * 128 - L_c
    # per-batch compacted keys + V-operand init image (zeros, with the
    # ones/denominator column set on live rows only)
    xk_b, vs_b, xb_b = [], [], []
    for b in range(B):
        idx = valid[b]
        xk = np.zeros((D, L_c), np.float32)
        xk[:, :len(idx)] = query[b][:, idx]
        xk_b.append(xk.astype(NPBF16))
        vs = np.zeros((MT, 128, HPC, 65), np.float32)
        for t, (ms, mw) in enumerate(mts):
            live = (ms + np.arange(128)) < len(idx)
            if t == MT - 1:
                live &= np.arange(128) >= deadw
            vs[t, :, :, 64] = live[:, None].astype(np.float32)
        vs_b.append(vs.astype(NPBF16))
        xb_b.append(query[b].astype(NPBF16))

    in_maps = []
    for c in range(NCORES):
        b, g = divmod(c, NCORES // B)
        sl = slice(g * DH, (g + 1) * DH)
        in_maps.append({
            "xb": xb_b[b],
            "xk": xk_b[b],
            "vsetup": vs_b[b],
            "wq": np.ascontiguousarray((Wq[sl, :] * scale).T).astype(NPBF16),
            "wk": np.ascontiguousarray(Wk[sl, :].T).astype(NPBF16),
            "wv": np.ascontiguousarray(Wv[sl, :].T).astype(NPBF16),
            "wo": np.ascontiguousarray(Wo[:, sl].T).astype(NPBF16),
            "bias": np.stack(
                [(bq[sl] * scale), bk[sl]], axis=-1
            ).reshape(2, 128, 2).transpose(1, 0, 2).reshape(128, 4).astype(np.float32),
        })

    nc = _get_nc(L_c)
    if TRACE:
        _install_ntff_hook()
    res = run_bass_kernel_spmd(nc, in_maps, core_ids=list(range(NCORES)), trace=TRACE)
    LAST_EXEC_NS = res.exec_time_ns
    LAST_RESULTS = res

    bo_eff = (Wo @ bv + bo)[:, None]   # bv folded through the O projection
    parts = [res.results[c]["out"] for c in range(NCORES)]
    for b in range(B):
        if len(valid[b]) == 0:
            out[b] = bo[:, None]
        else:
            acc = parts[4 * b].astype(np.float32)
            for g in range(1, 4):
                acc = acc + parts[4 * b + g]
            out[b] = acc + bo_eff
    return out
